# revision 2
# baseline (speedup 1.0000x reference)
"""Trainium2 Bass kernel v2 for ConvReverseDataNet (USRNet-style FFT data step).

Math per (b,c) plane (sf=2), storage X = Xr - i*Xs, 256x256 planes as [128,512]:
  g   = F128 x F128                       (fft2_128)
  FB  = G k G^T                           (G = F256[:, roll_idx])
  W   = sum_4blocks |FB|^2                (freq domain, squares + adds)
  Y0  = 4 * F~ b2 F~^T                    (spatial: b2 = 2x2-box-blurred psf, even samples,
                                           F~ = F128[:, (arange(13)-6)%128])
  wt  = s*(4 - Y0)/(W + 4*be), s = 1/65536
  u   = g * wt
  FXa = conj(FB) * tile(u)
  out = real(Fc FXa Fc) + s*real(E g E^T)   (E[m,up] = sum_a Fc[m,up+128a]*d[up+128a])
All matmuls in float32r (1 cyc/row at >=256-wide outputs).
Sharding: 256 (b,c) planes over 8 cores; core ci gets channels ci*8..ci*8+7 x 4 batches.
"""

import functools
import sys

import numpy as np

if "/opt/trn_rl_repo" not in sys.path:
    sys.path.insert(0, "/opt/trn_rl_repo")

from concourse import bacc, bass, mybir, tile  # noqa: E402
from concourse.bass_utils import run_bass_kernel_spmd  # noqa: E402

F32 = mybir.dt.float32
F32R = mybir.dt.float32r
MULT = mybir.AluOpType.mult
ADD = mybir.AluOpType.add
SUB = mybir.AluOpType.subtract

N_CORES = 8
NPL = 32
KS = 25
SCL = 1.0 / 65536.0


def _host_consts():
    t1 = np.arange(128)
    th1 = 2 * np.pi * np.outer(t1, t1) / 128
    C1 = np.cos(th1)
    S1 = np.sin(th1)
    t2 = np.arange(256)
    th2 = 2 * np.pi * np.outer(t2, t2) / 256
    C2 = np.cos(th2)
    S2 = np.sin(th2)
    idx25 = (np.arange(KS) - (KS // 2)) % 256
    GcT = C2[idx25, :]
    GsT = S2[idx25, :]
    # fft128 consts
    C1S1 = np.concatenate([C1, S1], axis=1)
    S1nC1 = np.concatenate([-S1, C1], axis=1)
    # Y0 spatial consts
    idx13 = (np.arange(13) - 6) % 128
    Ct = C1[:, idx13]  # [128,13]
    St = S1[:, idx13]
    F13a = 4.0 * np.concatenate([Ct.T, St.T], axis=1)  # [13,256]
    P25 = np.zeros((KS, 16))
    for a in range(13):
        P25[2 * a, a] = 1.0
        if 2 * a - 1 >= 0:
            P25[2 * a - 1, a] = 1.0
    # ifft natural-layout consts: Cnat[p, kc*256+m] = C2[kc*128+p, m]
    Cnat = C2.reshape(2, 128, 256).transpose(1, 0, 2).reshape(128, 512)
    Snat = S2.reshape(2, 128, 256).transpose(1, 0, 2).reshape(128, 512)
    CnS = np.concatenate(
        [Cnat[:, 0:256], -Snat[:, 0:256], Cnat[:, 256:512], -Snat[:, 256:512]], axis=1)
    SnC = np.concatenate(
        [Snat[:, 0:256], Cnat[:, 0:256], Snat[:, 256:512], Cnat[:, 256:512]], axis=1)
    # E matrix consts
    d = 1.0 + np.exp(-2j * np.pi * t2 / 256)
    Fc = np.exp(2j * np.pi * th2 / (2 * np.pi))  # e^{+i th2}
    Fc = np.cos(th2) + 1j * np.sin(th2)
    E = Fc[:, 0:128] * d[None, 0:128] + Fc[:, 128:256] * d[None, 128:256]  # [256,128]
    Er, Ei = np.real(E), np.imag(E)
    ETRIs = SCL * np.concatenate([Er.T, Ei.T], axis=1)      # [128,512]
    ETIRns = SCL * np.concatenate([Ei.T, -Er.T], axis=1)    # [128,512]
    EFr = np.ascontiguousarray(Er.T)                         # [128,256]
    EFin = np.ascontiguousarray(-Ei.T)                       # [128,256]
    c = {
        "C1S1": C1S1, "S1nC1": S1nC1,
        "GcT": GcT, "GsT": GsT, "GsTn": -GsT,
        "F13a": F13a, "Ft13c": np.ascontiguousarray(Ct.T),
        "Ft13s": np.ascontiguousarray(St.T), "Ft13sn": np.ascontiguousarray(-St.T),
        "P25": P25,
        "CnS": CnS, "SnC": SnC,
        "ETRIs": ETRIs, "ETIRns": ETIRns, "EFr": EFr, "EFin": EFin,
    }
    return {k: np.ascontiguousarray(v).astype(np.float32) for k, v in c.items()}


CONST_SHAPES = {
    "C1S1": [128, 256], "S1nC1": [128, 256],
    "GcT": [KS, 256], "GsT": [KS, 256], "GsTn": [KS, 256],
    "F13a": [13, 256], "Ft13c": [13, 128], "Ft13s": [13, 128], "Ft13sn": [13, 128],
    "P25": [KS, 16],
    "CnS": [128, 1024], "SnC": [128, 1024],
    "ETRIs": [128, 512], "ETIRns": [128, 512], "EFr": [128, 256], "EFin": [128, 256],
}


def _r(ap):
    return ap.bitcast(F32R)


def build_nc(n_planes=NPL):
    nc = bacc.Bacc("TRN2", target_bir_lowering=False, debug=False, num_devices=N_CORES)

    xs_t = nc.dram_tensor("xs", [n_planes, 128, 128], F32R, kind="ExternalInput")
    kt_t = nc.dram_tensor("kt", [n_planes, KS, KS], F32R, kind="ExternalInput")
    be4_t = nc.dram_tensor("be4", [128, n_planes], F32, kind="ExternalInput")
    const_t = {n: nc.dram_tensor(n, s, F32R, kind="ExternalInput") for n, s in CONST_SHAPES.items()}
    out_t = nc.dram_tensor("out", [n_planes, 256, 256], F32, kind="ExternalOutput")

    with tile.TileContext(nc) as tc:
        with (
            tc.tile_pool(name="cpool", bufs=1) as cpool,
            tc.tile_pool(name="small", bufs=3) as small,
            tc.tile_pool(name="big", bufs=3) as big,
            tc.tile_pool(name="psum", bufs=1, space="PSUM") as pp,
        ):
            cs = {}
            for n, s in CONST_SHAPES.items():
                cs[n] = cpool.tile(s, F32R, tag=n, name=f"c_{n}")
                nc.sync.dma_start(cs[n][:], const_t[n][:])
            be4sb = cpool.tile([128, n_planes], F32, tag="be4sb")
            nc.sync.dma_start(be4sb[:], be4_t[:])

            def front1(i):
                x_sb = small.tile([128, 128], F32R, tag="x_sb")
                nc.sync.dma_start(x_sb[:], xs_t[i])
                kt_sb = small.tile([KS, KS], F32R, tag="kt_sb")
                nc.sync.dma_start(kt_sb[:], kt_t[i])

                # fft128: g = F1 x F1  (pzg: [zA | g])
                pzg = pp.tile([128, 512], F32, tag="sh_zg_vt0")
                nc.tensor.matmul(pzg[:, 0:256], x_sb[:], cs["C1S1"][:], start=True, stop=True)
                zA = small.tile([128, 256], F32, tag="zA")
                nc.scalar.copy(_r(zA[:]), pzg[:, 0:256])
                nc.tensor.matmul(pzg[:, 256:512], _r(zA[:, 0:128]), cs["C1S1"][:], start=True, stop=False)
                nc.tensor.matmul(pzg[:, 256:512], _r(zA[:, 128:256]), cs["S1nC1"][:], start=False, stop=True)
                g_sb = small.tile([128, 256], F32, tag="g_sb")  # [gr | gs]
                nc.scalar.copy(_r(g_sb[:]), pzg[:, 256:512])

                # a = k-transform: [Ar | As] [25,512]
                papq = pp.tile([128, 512], F32, tag="sh_aq_vt1")
                nc.tensor.matmul(papq[0:KS, 0:256], kt_sb[:], cs["GcT"][:], start=True, stop=True)
                nc.tensor.matmul(papq[0:KS, 256:512], kt_sb[:], cs["GsT"][:], start=True, stop=True)
                a_sb = small.tile([KS, 512], F32, tag="a_sb")
                nc.scalar.copy(_r(a_sb[:]), papq[0:KS, :])

                # Y0 spatial: pm1 -> b2t -> pt13 -> py0
                pty = pp.tile([128, 512], F32, tag="pty")
                pm1 = pty[0:13, 0:32]
                nc.tensor.matmul(pm1[:, 0:KS], cs["P25"][:, 0:13].bitcast(F32), kt_sb[:].bitcast(F32),
                                 start=True, stop=True)
                m1s = small.tile([13, 32], F32, tag="m1s")
                nc.vector.tensor_copy(m1s[:], pm1[:])
                b2t = small.tile([13, 16], F32, tag="b2t")
                nc.vector.tensor_add(
                    _r(b2t[:, 1:13]),
                    m1s[:, 2:26].rearrange("p (i two) -> p two i", two=2)[:, 0, :],
                    m1s[:, 1:25].rearrange("p (i two) -> p two i", two=2)[:, 0, :],
                )
                nc.vector.tensor_copy(_r(b2t[:, 0:1]), m1s[:, 0:1])
                nc.tensor.matmul(pty[0:13, 0:256], _r(b2t[:, 0:13]), cs["F13a"][:], start=True, stop=True)
                t_sb = small.tile([13, 256], F32, tag="t_sb")
                nc.scalar.copy(_r(t_sb[:]), pty[0:13, 0:256])
                py0p = pty[:, 256:512]
                nc.tensor.matmul(py0p, cs["Ft13c"][:], _r(t_sb[:]), start=True, stop=False)
                nc.tensor.matmul(py0p[:, 0:128], cs["Ft13sn"][:], _r(t_sb[:, 128:256]), start=False, stop=True)
                nc.tensor.matmul(py0p[:, 128:256], cs["Ft13s"][:], _r(t_sb[:, 0:128]), start=False, stop=True)
                py0 = small.tile([128, 256], F32, tag="py0")
                nc.scalar.copy(py0[:], py0p)

                # FB: pfb_h = [FBr_h | FBs_h] [128,512], h=0,1
                pfb = []
                for hh in range(2):
                    hsl = slice(hh * 128, (hh + 1) * 128)
                    pf = pp.tile([128, 512], F32, tag=f"pfb{hh}", bufs=2)
                    nc.tensor.matmul(pf[:], cs["GcT"][:, hsl], _r(a_sb[:]), start=True, stop=False)
                    nc.tensor.matmul(pf[:, 0:256], cs["GsTn"][:, hsl], _r(a_sb[:, 256:512]), start=False, stop=True)
                    nc.tensor.matmul(pf[:, 256:512], cs["GsT"][:, hsl], _r(a_sb[:, 0:256]), start=False, stop=True)
                    pfb.append(pf)

                # E-part: pq = [Qr | Qi]
                pqt = pp.tile([128, 512], F32, tag="sh_aq_vt1")
                nc.tensor.matmul(pqt[:], _r(g_sb[:, 0:128]), cs["ETRIs"][:], start=True, stop=False)
                nc.tensor.matmul(pqt[:], _r(g_sb[:, 128:256]), cs["ETIRns"][:], start=False, stop=True)
                q_sb = big.tile([128, 512], F32, tag="q_sb")
                nc.vector.tensor_copy(_r(q_sb[:]), pqt[:])
                return g_sb, py0, pfb, q_sb

            def front2(i, g_sb, py0, pfb, q_sb):
                # W = sum |FB|^2 over 4 blocks
                sq0 = big.tile([128, 512], F32, tag="sq0")
                nc.scalar.square(sq0[:], pfb[0][:])
                sq1 = big.tile([128, 512], F32, tag="sq1")
                nc.scalar.square(sq1[:], pfb[1][:])
                r1 = big.tile([128, 512], F32, tag="r1")
                nc.gpsimd.tensor_add(r1[:], sq0[:], sq1[:])
                w2 = small.tile([128, 256], F32, tag="w2")
                v = r1[:].rearrange("p (a f) -> p a f", a=2)
                nc.gpsimd.tensor_add(w2[:], v[:, 0, :], v[:, 1, :])
                W = small.tile([128, 128], F32, tag="W")
                nc.gpsimd.tensor_add(W[:], w2[:, 0:128], w2[:, 128:256])

                # wt = s*(4 - Y0)/(W + 4be)
                den = small.tile([128, 128], F32, tag="den")
                nc.vector.tensor_scalar_add(den[:], W[:], be4sb[:, i:i + 1])
                dinv = small.tile([128, 128], F32, tag="dinv")
                nc.vector.reciprocal_approx_fast(dinv[:], den[:])
                wt4 = small.tile([128, 128], F32, tag="wt4")
                nc.vector.tensor_scalar(wt4[:], py0[:, 0:128], -1.0, 4.0, MULT, ADD)
                wt_sb = small.tile([128, 256], F32, tag="wt_sb")  # [wr | ws]
                nc.vector.scalar_tensor_tensor(wt_sb[:, 0:128], wt4[:], SCL, dinv[:], MULT, MULT)
                nc.vector.scalar_tensor_tensor(wt_sb[:, 128:256], py0[:, 128:256], -SCL, dinv[:], MULT, MULT)

                # u = g*wt
                e12 = small.tile([128, 256], F32, tag="e12")
                nc.vector.tensor_mul(e12[:], g_sb[:], wt_sb[:])
                e34 = small.tile([128, 256], F32, tag="e34")
                nc.vector.tensor_mul(e34[:, 0:128], g_sb[:, 128:256], wt_sb[:, 0:128])
                nc.vector.tensor_mul(e34[:, 128:256], g_sb[:, 0:128], wt_sb[:, 128:256])
                u_sb = small.tile([128, 256], F32, tag="u_sb")  # [ur | us]
                nc.vector.tensor_sub(u_sb[:, 0:128], e12[:, 0:128], e12[:, 128:256])
                nc.vector.tensor_add(u_sb[:, 128:256], e34[:, 0:128], e34[:, 128:256])

                # FXa = conj(FB)*tile(u)
                ub_r = u_sb[:, 0:128].unsqueeze(1).broadcast_to([128, 2, 128])
                ub_s = u_sb[:, 128:256].unsqueeze(1).broadcast_to([128, 2, 128])
                u4 = u_sb[:].rearrange("p (c f) -> p c f", c=2).unsqueeze(2).broadcast_to([128, 2, 2, 128])
                fx = []
                for hh in range(2):
                    pf = pfb[hh]
                    m1 = big.tile([128, 512], F32, tag=f"m1_{hh}")
                    nc.vector.tensor_tensor(
                        m1[:].rearrange("p (c d f) -> p c d f", c=2, d=2),
                        pf[:].rearrange("p (c d f) -> p c d f", c=2, d=2), u4, MULT)
                    m2 = big.tile([128, 512], F32, tag=f"m2_{hh}")
                    nc.vector.tensor_tensor(
                        m2[:, 0:256].rearrange("p (d f) -> p d f", d=2),
                        pf[:, 0:256].rearrange("p (d f) -> p d f", d=2), ub_s, MULT)
                    nc.vector.tensor_tensor(
                        m2[:, 256:512].rearrange("p (d f) -> p d f", d=2),
                        pf[:, 256:512].rearrange("p (d f) -> p d f", d=2), ub_r, MULT)
                    f = big.tile([128, 512], F32, tag=f"fx{hh}")  # [FXr_h | FXs_h]
                    nc.gpsimd.tensor_add(_r(f[:, 0:256]), m1[:, 0:256], m1[:, 256:512])
                    nc.gpsimd.tensor_sub(_r(f[:, 256:512]), m2[:, 0:256], m2[:, 256:512])
                    fx.append(f)
                return fx

            def back(i, fx, q_sb):
                vt = []
                for fb in range(2):
                    pv = pp.tile([128, 512], F32, tag=["sh_zg_vt0", "sh_aq_vt1"][fb])
                    for kc in range(2):
                        st = kc == 0
                        nc.tensor.matmul(pv[:], _r(fx[kc][:, fb * 128:(fb + 1) * 128]),
                                         cs["CnS"][:, kc * 512:(kc + 1) * 512], start=st, stop=False)
                        nc.tensor.matmul(pv[:], _r(fx[kc][:, 256 + fb * 128:256 + (fb + 1) * 128]),
                                         cs["SnC"][:, kc * 512:(kc + 1) * 512], start=False, stop=(kc == 1))
                    v_sb = big.tile([128, 512], F32, tag=f"vt{fb}")
                    nc.scalar.copy(_r(v_sb[:]), pv[:])
                    vt.append(v_sb)

                po = pp.tile([128, 512], F32, tag="po")
                for mb in range(2):
                    osl = slice(mb * 256, (mb + 1) * 256)
                    msl = slice(mb * 128, (mb + 1) * 128)
                    msl2 = slice(256 + mb * 128, 256 + (mb + 1) * 128)
                    nc.tensor.matmul(po[:, osl], _r(vt[0][:, msl]), cs["CnS"][:, 0:256], start=True, stop=False)
                    nc.tensor.matmul(po[:, osl], _r(vt[0][:, msl2]), cs["SnC"][:, 0:256], start=False, stop=False)
                    nc.tensor.matmul(po[:, osl], _r(vt[1][:, msl]), cs["CnS"][:, 512:768], start=False, stop=False)
                    nc.tensor.matmul(po[:, osl], _r(vt[1][:, msl2]), cs["SnC"][:, 512:768], start=False, stop=False)
                    nc.tensor.matmul(po[:, osl], _r(q_sb[:, msl]), cs["EFr"][:], start=False, stop=False)
                    nc.tensor.matmul(po[:, osl], _r(q_sb[:, msl2]), cs["EFin"][:], start=False, stop=True)
                out_sb = big.tile([128, 512], F32, tag="out_sb")
                nc.scalar.copy(out_sb[:], po[:])
                nc.sync.dma_start(
                    out_t[i].rearrange("(hb p) f -> p hb f", p=128),
                    out_sb[:].rearrange("p (hb f) -> p hb f", hb=2),
                )

            # 3-stage pipeline: back(i-2) first (ready work, PE burst), then
            # front1(i) (PE-heavy), then front2(i-1) (DVE/Act/Pool chain).
            st1 = {}  # i -> (g, py0, pfb, q)
            st2 = {}  # i -> fx
            for t in range(n_planes + 2):
                if t >= 2:
                    i2 = t - 2
                    back(i2, st2.pop(i2), st1.pop(i2)[3])
                if t < n_planes:
                    st1[t] = front1(t)
                if t >= 1 and t - 1 < n_planes:
                    i1 = t - 1
                    g_sb, py0, pfb, q_sb = st1[i1]
                    st2[i1] = front2(i1, g_sb, py0, pfb, q_sb)

    nc.compile()
    return nc


@functools.lru_cache(maxsize=2)
def _built(n_planes=NPL):
    return build_nc(n_planes)


def make_in_maps(x, k, alpha, n_planes=NPL, n_cores=N_CORES):
    consts = _host_consts()
    alpha_c = alpha.reshape(-1).astype(np.float64)
    be = (1.0 / (1.0 + np.exp(-(alpha_c - 9.0))) + 1e-3).astype(np.float32)
    cpc = n_planes // 4
    in_maps = []
    for ci in range(n_cores):
        chs = slice(ci * cpc, (ci + 1) * cpc)
        xs = np.ascontiguousarray(x[:, chs].transpose(1, 0, 2, 3).reshape(n_planes, 128, 128))
        kt = np.ascontiguousarray(k[:, chs].transpose(1, 0, 3, 2).reshape(n_planes, KS, KS))
        be_pl = np.repeat(be[chs], 4)
        be4 = np.broadcast_to(4.0 * be_pl, (128, n_planes)).astype(np.float32).copy()
        m = {"xs": xs, "kt": kt, "be4": be4}
        m.update(consts)
        in_maps.append(m)
    return in_maps


def kernel(x, k, alpha, sf=2, **_ignored):
    x = np.asarray(x, dtype=np.float32)
    k = np.asarray(k, dtype=np.float32)
    alpha = np.asarray(alpha, dtype=np.float32)
    assert int(sf) == 2 and x.shape == (4, 64, 128, 128) and k.shape == (4, 64, KS, KS)

    nc = _built(NPL)
    in_maps = make_in_maps(x, k, alpha)
    res = run_bass_kernel_spmd(nc, in_maps, core_ids=list(range(N_CORES)))
    out = np.empty((4, 64, 256, 256), np.float32)
    cpc = NPL // 4
    for ci in range(N_CORES):
        o = res.results[ci]["out"].reshape(cpc, 4, 256, 256).transpose(1, 0, 2, 3)
        out[:, ci * cpc:(ci + 1) * cpc] = o
    return out


if __name__ == "__main__":
    rng = np.random.default_rng(0)
    x = rng.standard_normal((4, 64, 128, 128), dtype=np.float32)
    k = rng.random((4, 64, KS, KS), dtype=np.float32)
    alpha = np.zeros((1, 64, 1, 1), np.float32)
    out = kernel(x, k, alpha, 2)
    print("out", out.shape, out.dtype, float(np.abs(out).max()))
